# revision 44
# baseline (speedup 1.0000x reference)
"""Causal multi-head attention (B=2, T=2048, C=1024, H=16, d=64) on 8 trn2 cores.

Sharding: core i -> (batch b = i//4, head group g = i%4, 4 heads/core).
Data parallel over B, tensor parallel over heads; the out-proj partial sums
(contraction over this core's 256 channels) are reduced on the host during
the gather step, along with b_proj and the analytically-folded V bias.

Device kernel works entirely in [feature, token] (transposed) layout so no
on-device transposes are needed:
  stage 1: Q^T,K^T = (Wqk)^T x^T   (+bias, 1/sqrt(d) folded into Wq host-side)
           V       = x W_v         (natural layout, used as stage-4 lhsT)
  stage 2: S^T[j,q] = K_h^T.T @ Q_h^T   per head, causal tiles only
  stage 3: P^T = exp(S^T + mask)        (no max subtraction; scores are O(3))
  stage 4: outT[65,q] = [V_h | 1]^T.T @ P^T  accumulated over j tiles
           (row 64 = softmax denominator Z)
  stage 5: att^T = outT[0:64] * (1/Z) via reciprocal_approx_fast (DVE) +
           partition_broadcast (GPSIMD)
  stage 6: y^T = Wp.T @ att^T  -> DMA out; host sums partials + transposes.

All matmul operands are bfloat16 (full PE rate, fast weight load, half the
SBUF/HBM traffic and PE power of fp32r -> no HAM power throttle); PSUM
accumulation stays fp32.

Schedule: the ScalarE exp stream (~90us) is the long pole of the attention
phase, so the PE work that does NOT feed ACT is woven INTO the attention
phase as filler: preamble computes only V and Q/K for head-pair 0, then
pair-0 attention carries the pair-1 Q/K projection matmuls as fillers and
pair-1 attention carries the out-proj matmuls as fillers.  Input DMA is
split into 512-column chunks so the first projection matmuls start ~3us in.
"""

from collections import deque

import numpy as np

import concourse.bass as bass
import concourse.mybir as mybir
from concourse import bacc
import concourse.tile as tile
from concourse.bass_utils import run_bass_kernel_spmd

B, T, C, H, D = 2, 2048, 1024, 16, 64
NCORES = 8
HPC = 4            # heads per core
CS = HPC * D       # 256 channels per core (per Q/K/V block)
KT = C // 128      # 8 contraction tiles for the projections
NT = T // 128      # 16 token tiles of 128
QB = 512           # query block (psum bank width in fp32)
NQB = T // QB      # 4 query blocks
NEG = -1e9

F32 = mybir.dt.float32
BF16 = mybir.dt.bfloat16

TRACE = False
LAST_RESULT = None


def _build_body(nc, tc, ctx, xT, wqkv, bqk, wp, masks, yT):
    AF = mybir.ActivationFunctionType

    persist = ctx.enter_context(tc.tile_pool(name="persist", bufs=1))

    wqkv_sb = [persist.tile([128, 3 * CS], BF16, tag=f"wqkv{k}", name=f"wqkv{k}") for k in range(KT)]
    bqk_sb = persist.tile([128, 4], F32, tag="bqk", name="bqk_sb")
    wp_sb = [persist.tile([128, C], BF16, tag=f"wp{k}", name=f"wp{k}") for k in range(2)]
    mask_sb = persist.tile([128, 128], BF16, tag="mask", name="mask_sb")
    qT_sb = [persist.tile([128, T], BF16, tag=f"qT{i}", name=f"qT{i}") for i in range(2)]
    kT_sb = [persist.tile([128, T], BF16, tag=f"kT{i}", name=f"kT{i}") for i in range(2)]
    v_sb = [persist.tile([128, HPC, D + 1], BF16, tag=f"v{t}", name=f"v{t}") for t in range(NT)]
    attT_sb = [persist.tile([128, T], BF16, tag=f"attT{i}", name=f"attT{i}") for i in range(2)]
    xT_sb = [persist.tile([128, T], BF16, tag=f"xT{k}", name=f"xT{k}") for k in range(KT)]

    # PSUM: sT 2x2-bank + oT 2x1-bank + fill 2x1-bank = 8 banks
    sTp = ctx.enter_context(tc.tile_pool(name="sT", bufs=2, space="PSUM"))
    oTp = ctx.enter_context(tc.tile_pool(name="outT", bufs=2, space="PSUM"))
    fillp = ctx.enter_context(tc.tile_pool(name="fill", bufs=2, space="PSUM"))
    pTp = ctx.enter_context(tc.tile_pool(name="pT", bufs=8))
    smallp = ctx.enter_context(tc.tile_pool(name="small", bufs=8))
    ysp = ctx.enter_context(tc.tile_pool(name="ystage", bufs=10))

    # ---- DMA: each dma_start costs ~650ns of serial SyncE trigger time, so
    # minimize count and put the critical (xT[k], wqkv[k]) pairs first ----
    for k in range(KT):
        nc.sync.dma_start(out=xT_sb[k][:, :], in_=xT[k * 128:(k + 1) * 128, :])
        nc.sync.dma_start(out=wqkv_sb[k][:, :], in_=wqkv[k * 128:(k + 1) * 128, :])
    nc.sync.dma_start(out=bqk_sb[:, :], in_=bqk[:, :])
    nc.sync.dma_start(out=mask_sb[:, :], in_=masks[:, :])
    for k in range(2):
        nc.sync.dma_start(out=wp_sb[k][:, :], in_=wp[k * 128:(k + 1) * 128, :])

    ones_f32 = smallp.tile([128, 4], F32, tag="ones_f32", name="ones_f32")
    nc.vector.memset(ones_f32[:, :], 1.0)
    for t in range(NT):
        nc.vector.tensor_copy(v_sb[t][:, :, D], ones_f32[:, :])

    def emit_qk_half(ct, tc4, half, ps):
        # half a k-sweep (4 of 8 contraction tiles) of one Q/K 128x512 block
        for k in range(half * 4, half * 4 + 4):
            nc.tensor.matmul(
                ps[:, :],
                lhsT=wqkv_sb[k][:, ct * 128:(ct + 1) * 128],
                rhs=xT_sb[k][:, tc4 * QB:(tc4 + 1) * QB],
                start=(k == 0),
                stop=(k == KT - 1),
            )
        if half == 1:
            dst = qT_sb[ct] if ct < 2 else kT_sb[ct - 2]
            nc.vector.tensor_scalar_add(
                dst[:, tc4 * QB:(tc4 + 1) * QB], ps[:, :], bqk_sb[:, ct:ct + 1]
            )

    def emit_v(t):
        ps = fillp.tile([128, QB], F32, tag="fps", name="fps")
        for k in range(KT):
            nc.tensor.matmul(
                ps[:, 0:CS],
                lhsT=xT_sb[k][:, t * 128:(t + 1) * 128],
                rhs=wqkv_sb[k][:, 2 * CS:3 * CS],
                start=(k == 0),
                stop=(k == KT - 1),
            )
        nc.vector.tensor_copy(
            v_sb[t][:, :, 0:D],
            ps[:, 0:CS].rearrange("p (h d) -> p h d", h=HPC),
        )

    def mk_qk(ct, tc4):
        def thunk():
            ps = fillp.tile([128, QB], F32, tag="fps", name="fps")
            emit_qk_half(ct, tc4, 0, ps)
            emit_qk_half(ct, tc4, 1, ps)
        return [thunk]

    def mk_v(t):
        def thunk():
            emit_v(t)
        return [thunk]

    # ---- preamble A: just enough for the qb<=1 attention units: Q/K pair 0
    # for tc4 0,1 + V t0-7.  The four Q/K sweeps interleave their k-loops so
    # each arriving (xT[k], wqkv[k]) DMA pair feeds four back-to-back
    # matmuls: the PE never idles waiting for the next k-tile, so the HAM
    # clock-gate warms up during the DMA-paced phase instead of after it.
    pre_sweeps = [(0, 1), (2, 1), (0, 0), (2, 0)]
    pre_ps = [
        fillp.tile([128, QB], F32, tag="fps", name="fps"),
        fillp.tile([128, QB], F32, tag="fps", name="fps"),
        oTp.tile([128, QB], F32, tag="oT", name="oT"),
        oTp.tile([128, QB], F32, tag="oT", name="oT"),
    ]
    for k in range(KT):
        for s, (ct, tc4) in enumerate(pre_sweeps):
            nc.tensor.matmul(
                pre_ps[s][:, :],
                lhsT=wqkv_sb[k][:, ct * 128:(ct + 1) * 128],
                rhs=xT_sb[k][:, tc4 * QB:(tc4 + 1) * QB],
                start=(k == 0),
                stop=(k == KT - 1),
            )
    for s, (ct, tc4) in enumerate(pre_sweeps):
        dst = qT_sb[ct] if ct < 2 else kT_sb[ct - 2]
        nc.vector.tensor_scalar_add(
            dst[:, tc4 * QB:(tc4 + 1) * QB], pre_ps[s][:, :], bqk_sb[:, ct:ct + 1]
        )
    for t in range(8):
        emit_v(t)

    # ---- fillers woven into the attention phase ----
    # preamble B rides as filler during the qb=1 units: K^T pair 0 for tc4
    # 3,2 first (the qb=3 S-matmuls sweep every j-tile), then qT tc4=3 and
    # V t8-15 (qb=3 O-matmuls), then qT tc4=2 (only qb=2 needs it)
    fillers = deque()
    for ct, tc4 in ((2, 3), (2, 2), (0, 3)):
        fillers.extend(mk_qk(ct, tc4))
    for t in range(8, NT):
        fillers.extend(mk_v(t))
    fillers.extend(mk_qk(0, 2))

    # out-proj: two q-blocks share one ys staging tile + one output DMA
    # (fewer SyncE triggers, 2KB DMA lines).  qb descends, so slot qb-base+0
    # is written second and fires the DMA.
    ys_hold = {}

    def mk_outproj(sqb, et):
        def thunk():
            yps = fillp.tile([128, QB], F32, tag="fps", name="fps")
            for kc in range(2):
                nc.tensor.matmul(
                    yps[:, :],
                    lhsT=wp_sb[kc][:, et * 128:(et + 1) * 128],
                    rhs=attT_sb[kc][:, sqb * QB:(sqb + 1) * QB],
                    start=(kc == 0),
                    stop=(kc == 1),
                )
            base = 2 if sqb >= 2 else 0
            if sqb - base == 1:
                ys_hold[(et, base)] = ysp.tile([128, 2, QB], BF16, tag="ys", name="ys")
            ys = ys_hold[(et, base)]
            if sqb <= 1:
                # tail q-blocks: the exp stream is nearly done, ScalarE has
                # slack -- use it for the PSUM->SBUF copy so the DVE isn't
                # the serializer of the out-proj drain
                nc.scalar.copy(ys[:, sqb - base, :], yps[:, :])
            else:
                nc.vector.tensor_copy(ys[:, sqb - base, :], yps[:, :])
            if sqb == base:
                nc.sync.dma_start(
                    out=yT[et * 128:(et + 1) * 128, base * QB:(base + 2) * QB],
                    in_=ys[:, :, :],
                )
        return thunk

    gctr = [0]
    pace = [1]

    def pop_filler():
        gctr[0] += 1
        pops = 1 if gctr[0] % pace[0] == 0 else 0
        if len(fillers) > 6:
            pops += 1
        if len(fillers) > 10:
            pops += 1
        for _ in range(pops):
            if fillers:
                fillers.popleft()()

    # ---- attention unit for one (head, q-block) ----
    # Each unit's O-matmul flush + softmax normalization is returned as a
    # closure and emitted inside the NEXT unit right after its first exp, so
    # ACT gets the next unit's first exp with no boundary gap.
    def attention(h, qb, leftover):
        pair = h // 2
        ktile, qtile = kT_sb[pair], qT_sb[pair]
        po = (h % 2) * D
        ngr = 2 * (qb + 1)   # groups of 2 j-tiles each
        njt = 4 * (qb + 1)
        oT = oTp.tile([128, QB], F32, tag="oT", name="oT")

        def emit_omms(grp, pT):
            for m in range(2):
                jt = grp * 2 + m
                c0 = 128 * (jt - 4 * qb) if grp >= ngr - 2 else 0
                nc.tensor.matmul(
                    oT[0:D + 1, c0:QB],
                    lhsT=v_sb[jt][:, h, :],
                    rhs=pT[:, m, c0:QB],
                    start=(jt == 0),
                    stop=(jt == njt - 1),
                )

        pend = []
        for grp in range(ngr):
            diag = grp >= ngr - 2
            sT = sTp.tile([128, 2, QB], F32, tag="sT", name="sT")
            for m in range(2):
                jt = grp * 2 + m
                c0 = 128 * (jt - 4 * qb) if diag else 0
                nc.tensor.matmul(
                    sT[:, m, c0:QB],
                    lhsT=ktile[po:po + D, jt * 128:(jt + 1) * 128],
                    rhs=qtile[po:po + D, qb * QB + c0:(qb + 1) * QB],
                    start=True,
                    stop=True,
                )
            pT = pTp.tile([128, 2, QB], BF16, tag="pT", name="pT")
            if diag:
                # one ACT call covering both m; m1's first 128 cols are
                # garbage-exp'd but never read by the O-matmuls
                c0 = 128 * (grp * 2 - 4 * qb)
                nc.scalar.activation(pT[:, :, c0:QB], sT[:, :, c0:QB], AF.Exp)
                # multiplicative causal mask on the diagonal 128-blocks of
                # P = exp(S), applied AFTER exp so the DVE never sits between
                # S-matmul and exp (the O-matmuls trail by 2 groups anyway)
                for m in range(2):
                    jt = grp * 2 + m
                    cm = 128 * (jt - 4 * qb)
                    nc.vector.tensor_mul(
                        pT[:, m, cm:cm + 128], pT[:, m, cm:cm + 128], mask_sb[:, :]
                    )
            else:
                nc.scalar.activation(pT[:, :, :], sT[:, :, :], AF.Exp)
            pend.append((grp, pT))
            if grp == 0 and leftover is not None:
                leftover()
            else:
                pop_filler()
            if len(pend) > 2:
                emit_omms(*pend.pop(0))

        def fin():
            for g0, p0 in pend:
                emit_omms(g0, p0)
            # normalize: att^T = outT[0:D] * (1/Z), Z = outT[D]
            zrow = smallp.tile([1, QB], F32, tag="zrow", name="zrow")
            nc.vector.tensor_copy(zrow[:, :], oT[D:D + 1, :])
            rz = smallp.tile([1, QB], F32, tag="rz", name="rz")
            nc.vector.reciprocal_approx_fast(out=rz[:, :], in_=zrow[:, :])
            zs = smallp.tile([D, QB], F32, tag="zs", name="zs")
            nc.gpsimd.partition_broadcast(zs[:, :], rz[:, :], channels=D)
            nc.vector.tensor_mul(
                attT_sb[pair][po:po + D, qb * QB:(qb + 1) * QB],
                oT[0:D, :],
                zs[:, :],
            )
        return fin

    # pair-0 qb=1 units first (they only need preamble A) with preamble B as
    # filler; the tiny qb=0 units come last so the meaty units carry the
    # early phase
    fin = None
    fin = attention(0, 1, fin)
    fin = attention(1, 1, fin)
    # pair-1 Q/K projection fillers: all of K^T first (every pair-1 S-matmul
    # needs full kT), then Q^T in the pair-1 qb visit order
    for tc4 in range(NQB):
        fillers.extend(mk_qk(3, tc4))
    for tc4 in (3, 2, 1, 0):
        fillers.extend(mk_qk(1, tc4))
    pace[0] = 3
    for qb in (3, 2, 0):
        fin = attention(0, qb, fin)
        fin = attention(1, qb, fin)
    # pair 1 attention carries the out-proj as filler
    pace[0] = 2
    for qb in (3, 2, 1, 0):
        fin = attention(2, qb, fin)
        fin = attention(3, qb, fin)
        for et in range(C // 128):
            fillers.append(mk_outproj(qb, et))
    fin()
    while fillers:
        fillers.popleft()()


def build_nc():
    from contextlib import ExitStack

    nc = bacc.Bacc("TRN2", target_bir_lowering=False)
    xT = nc.dram_tensor("xT", [C, T], BF16, kind="ExternalInput")
    wqkv = nc.dram_tensor("wqkv", [C, 3 * CS], BF16, kind="ExternalInput")
    bqk = nc.dram_tensor("bqk", [128, 4], F32, kind="ExternalInput")
    wp = nc.dram_tensor("wp", [CS, C], BF16, kind="ExternalInput")
    masks = nc.dram_tensor("masks", [128, 128], BF16, kind="ExternalInput")
    yT = nc.dram_tensor("yT", [C, T], BF16, kind="ExternalOutput")
    with tile.TileContext(nc) as tc:
        with nc.allow_low_precision(reason="bf16 matmul inputs; accumulation stays fp32 in PSUM"):
            with ExitStack() as ctx:
                _build_body(nc, tc, ctx, xT, wqkv, bqk, wp, masks, yT)
    nc.compile()
    return nc


def make_masks():
    from ml_dtypes import bfloat16

    r = np.arange(128)[:, None]
    c = np.arange(128)[None, :]
    return np.where(r <= c, 1.0, 0.0).astype(bfloat16)


def make_in_maps(x, W_qkv, b_qkv, W_proj):
    from ml_dtypes import bfloat16

    scale = np.float32(1.0 / np.sqrt(D))
    mask_h = make_masks()
    xT_b = [np.ascontiguousarray(x[b].T).astype(bfloat16) for b in range(B)]
    in_maps = []
    for i in range(NCORES):
        b, g = divmod(i, HPC)
        cs0 = g * CS
        wq = W_qkv[:, cs0:cs0 + CS] * scale
        wk = W_qkv[:, C + cs0:C + cs0 + CS]
        wv = W_qkv[:, 2 * C + cs0:2 * C + cs0 + CS]
        bq = b_qkv[cs0:cs0 + CS] * scale
        bk = b_qkv[C + cs0:C + cs0 + CS]
        in_maps.append({
            "xT": xT_b[b],
            "wqkv": np.concatenate([wq, wk, wv], axis=1).astype(bfloat16),
            "bqk": np.ascontiguousarray(
                np.concatenate([bq, bk]).reshape(4, 128).T
            ).astype(np.float32),
            "wp": np.ascontiguousarray(W_proj[cs0:cs0 + CS, :]).astype(bfloat16),
            "masks": mask_h,
        })
    return in_maps


_NC_CACHE = None


def _get_nc():
    global _NC_CACHE
    if _NC_CACHE is None:
        _NC_CACHE = build_nc()
    return _NC_CACHE


def gather(results, b_qkv, W_proj, b_proj):
    Y = np.zeros((B, T, C), np.float32)
    for i in range(NCORES):
        Y[i // HPC] += results[i]["yT"].astype(np.float32).T
    Y += (b_qkv[2 * C:].astype(np.float32) @ W_proj.astype(np.float32)
          + b_proj.astype(np.float32))[None, None, :]
    return Y


def kernel(x, W_qkv, b_qkv, W_proj, b_proj):
    global LAST_RESULT
    x = np.asarray(x, np.float32)
    W_qkv = np.asarray(W_qkv, np.float32)
    b_qkv = np.asarray(b_qkv, np.float32)
    W_proj = np.asarray(W_proj, np.float32)
    b_proj = np.asarray(b_proj, np.float32)

    nc = _get_nc()
    in_maps = make_in_maps(x, W_qkv, b_qkv, W_proj)
    res = run_bass_kernel_spmd(nc, in_maps, list(range(NCORES)), trace=TRACE)
    LAST_RESULT = res
    if TRACE and res.exec_time_ns is not None:
        print(f"HW exec time: {res.exec_time_ns} ns")
    return gather(res.results, b_qkv, W_proj, b_proj)


# revision 46
# speedup vs baseline: 1.0245x; 1.0245x over previous
"""Causal multi-head attention (B=2, T=2048, C=1024, H=16, d=64) on 8 trn2 cores.

Sharding: core i -> (batch b = i//4, head group g = i%4, 4 heads/core).
Data parallel over B, tensor parallel over heads; the out-proj partial sums
(contraction over this core's 256 channels) are reduced on the host during
the gather step, along with b_proj and the analytically-folded V bias.

Device kernel works entirely in [feature, token] (transposed) layout so no
on-device transposes are needed:
  stage 1: Q^T,K^T = (Wqk)^T x^T   (+bias, 1/sqrt(d) folded into Wq host-side)
           V       = x W_v         (natural layout, used as stage-4 lhsT)
  stage 2: S^T[j,q] = K_h^T.T @ Q_h^T   per head, causal tiles only
  stage 3: P^T = exp(S^T + mask)        (no max subtraction; scores are O(3))
  stage 4: outT[65,q] = [V_h | 1]^T.T @ P^T  accumulated over j tiles
           (row 64 = softmax denominator Z)
  stage 5: att^T = outT[0:64] * (1/Z) via reciprocal_approx_fast (DVE) +
           partition_broadcast (GPSIMD)
  stage 6: y^T = Wp.T @ att^T  -> DMA out; host sums partials + transposes.

All matmul operands are bfloat16 (full PE rate, fast weight load, half the
SBUF/HBM traffic and PE power of fp32r -> no HAM power throttle); PSUM
accumulation stays fp32.

Schedule: the ScalarE exp stream (~90us) is the long pole of the attention
phase, so the PE work that does NOT feed ACT is woven INTO the attention
phase as filler: preamble computes only V and Q/K for head-pair 0, then
pair-0 attention carries the pair-1 Q/K projection matmuls as fillers and
pair-1 attention carries the out-proj matmuls as fillers.  Input DMA is
split into 512-column chunks so the first projection matmuls start ~3us in.
"""

from collections import deque

import numpy as np

import concourse.bass as bass
import concourse.mybir as mybir
from concourse import bacc
import concourse.tile as tile
from concourse.bass_utils import run_bass_kernel_spmd

B, T, C, H, D = 2, 2048, 1024, 16, 64
NCORES = 8
HPC = 4            # heads per core
CS = HPC * D       # 256 channels per core (per Q/K/V block)
KT = C // 128      # 8 contraction tiles for the projections
NT = T // 128      # 16 token tiles of 128
QB = 512           # query block (psum bank width in fp32)
NQB = T // QB      # 4 query blocks
NEG = -1e9

F32 = mybir.dt.float32
BF16 = mybir.dt.bfloat16

TRACE = False
LAST_RESULT = None


def _build_body(nc, tc, ctx, xT, wqkv, bqk, wp, masks, yT):
    AF = mybir.ActivationFunctionType

    persist = ctx.enter_context(tc.tile_pool(name="persist", bufs=1))

    wqkv_sb = [persist.tile([128, 3 * CS], BF16, tag=f"wqkv{k}", name=f"wqkv{k}") for k in range(KT)]
    bqk_sb = persist.tile([128, 4], F32, tag="bqk", name="bqk_sb")
    wp_sb = [persist.tile([128, C], BF16, tag=f"wp{k}", name=f"wp{k}") for k in range(2)]
    mask_sb = persist.tile([128, 128], BF16, tag="mask", name="mask_sb")
    qT_sb = [persist.tile([128, T], BF16, tag=f"qT{i}", name=f"qT{i}") for i in range(2)]
    kT_sb = [persist.tile([128, T], BF16, tag=f"kT{i}", name=f"kT{i}") for i in range(2)]
    v_sb = [persist.tile([128, HPC, D + 1], BF16, tag=f"v{t}", name=f"v{t}") for t in range(NT)]
    attT_sb = [persist.tile([128, T], BF16, tag=f"attT{i}", name=f"attT{i}") for i in range(2)]
    xT_sb = [persist.tile([128, T], BF16, tag=f"xT{k}", name=f"xT{k}") for k in range(KT)]

    # PSUM: sT 2x2-bank + oT 2x1-bank + fill 2x1-bank = 8 banks
    sTp = ctx.enter_context(tc.tile_pool(name="sT", bufs=2, space="PSUM"))
    oTp = ctx.enter_context(tc.tile_pool(name="outT", bufs=2, space="PSUM"))
    fillp = ctx.enter_context(tc.tile_pool(name="fill", bufs=2, space="PSUM"))
    pTp = ctx.enter_context(tc.tile_pool(name="pT", bufs=8))
    smallp = ctx.enter_context(tc.tile_pool(name="small", bufs=8))
    ysp = ctx.enter_context(tc.tile_pool(name="ystage", bufs=10))

    # ---- DMA: each dma_start costs ~650ns of serial SyncE trigger time, so
    # minimize count and put the critical (xT[k], wqkv[k]) pairs first ----
    for k in range(KT):
        nc.sync.dma_start(out=xT_sb[k][:, :], in_=xT[k * 128:(k + 1) * 128, :])
        nc.sync.dma_start(out=wqkv_sb[k][:, :], in_=wqkv[k * 128:(k + 1) * 128, :])
    nc.sync.dma_start(out=bqk_sb[:, :], in_=bqk[:, :])
    nc.sync.dma_start(out=mask_sb[:, :], in_=masks[:, :])
    for k in range(2):
        nc.sync.dma_start(out=wp_sb[k][:, :], in_=wp[k * 128:(k + 1) * 128, :])

    ones_f32 = smallp.tile([128, 4], F32, tag="ones_f32", name="ones_f32")
    nc.vector.memset(ones_f32[:, :], 1.0)
    for t in range(NT):
        nc.vector.tensor_copy(v_sb[t][:, :, D], ones_f32[:, :])

    def emit_qk_half(ct, tc4, half, ps):
        # half a k-sweep (4 of 8 contraction tiles) of one Q/K 128x512 block
        for k in range(half * 4, half * 4 + 4):
            nc.tensor.matmul(
                ps[:, :],
                lhsT=wqkv_sb[k][:, ct * 128:(ct + 1) * 128],
                rhs=xT_sb[k][:, tc4 * QB:(tc4 + 1) * QB],
                start=(k == 0),
                stop=(k == KT - 1),
            )
        if half == 1:
            dst = qT_sb[ct] if ct < 2 else kT_sb[ct - 2]
            nc.vector.tensor_scalar_add(
                dst[:, tc4 * QB:(tc4 + 1) * QB], ps[:, :], bqk_sb[:, ct:ct + 1]
            )

    def emit_v(t):
        ps = fillp.tile([128, QB], F32, tag="fps", name="fps")
        for k in range(KT):
            nc.tensor.matmul(
                ps[:, 0:CS],
                lhsT=xT_sb[k][:, t * 128:(t + 1) * 128],
                rhs=wqkv_sb[k][:, 2 * CS:3 * CS],
                start=(k == 0),
                stop=(k == KT - 1),
            )
        nc.vector.tensor_copy(
            v_sb[t][:, :, 0:D],
            ps[:, 0:CS].rearrange("p (h d) -> p h d", h=HPC),
        )

    def mk_qk(ct, tc4):
        def thunk():
            ps = fillp.tile([128, QB], F32, tag="fps", name="fps")
            emit_qk_half(ct, tc4, 0, ps)
            emit_qk_half(ct, tc4, 1, ps)
        return [thunk]

    def mk_v(t):
        def thunk():
            emit_v(t)
        return [thunk]

    # ---- preamble A: just enough for the qb<=1 attention units: Q/K pair 0
    # for tc4 0,1 + V t0-7.  The four Q/K sweeps interleave their k-loops so
    # each arriving (xT[k], wqkv[k]) DMA pair feeds four back-to-back
    # matmuls: the PE never idles waiting for the next k-tile, so the HAM
    # clock-gate warms up during the DMA-paced phase instead of after it.
    pre_sweeps = [(0, 1), (2, 1), (0, 0), (2, 0)]
    pre_ps = [
        fillp.tile([128, QB], F32, tag="fps", name="fps"),
        fillp.tile([128, QB], F32, tag="fps", name="fps"),
        oTp.tile([128, QB], F32, tag="oT", name="oT"),
        oTp.tile([128, QB], F32, tag="oT", name="oT"),
    ]
    for k in range(KT):
        for s, (ct, tc4) in enumerate(pre_sweeps):
            nc.tensor.matmul(
                pre_ps[s][:, :],
                lhsT=wqkv_sb[k][:, ct * 128:(ct + 1) * 128],
                rhs=xT_sb[k][:, tc4 * QB:(tc4 + 1) * QB],
                start=(k == 0),
                stop=(k == KT - 1),
            )
    for s, (ct, tc4) in enumerate(pre_sweeps):
        dst = qT_sb[ct] if ct < 2 else kT_sb[ct - 2]
        nc.vector.tensor_scalar_add(
            dst[:, tc4 * QB:(tc4 + 1) * QB], pre_ps[s][:, :], bqk_sb[:, ct:ct + 1]
        )
    for t in range(8):
        emit_v(t)

    # ---- fillers woven into the attention phase ----
    # preamble B rides as filler during the qb<=1 units: Q/K pair 0 for
    # tc4 3,2 (3 first: the qb=3 units need it next) + V t8-15
    fillers = deque()
    for tc4 in (3, 2):
        for ct in (0, 2):
            fillers.extend(mk_qk(ct, tc4))
    for t in range(8, NT):
        fillers.extend(mk_v(t))

    # out-proj: two q-blocks share one ys staging tile + one output DMA
    # (fewer SyncE triggers, 2KB DMA lines).  qb descends, so slot qb-base+0
    # is written second and fires the DMA.
    ys_hold = {}

    def mk_outproj(sqb, et):
        def thunk():
            yps = fillp.tile([128, QB], F32, tag="fps", name="fps")
            for kc in range(2):
                nc.tensor.matmul(
                    yps[:, :],
                    lhsT=wp_sb[kc][:, et * 128:(et + 1) * 128],
                    rhs=attT_sb[kc][:, sqb * QB:(sqb + 1) * QB],
                    start=(kc == 0),
                    stop=(kc == 1),
                )
            base = 2 if sqb >= 2 else 0
            if sqb - base == 1:
                ys_hold[(et, base)] = ysp.tile([128, 2, QB], BF16, tag="ys", name="ys")
            ys = ys_hold[(et, base)]
            if sqb <= 1:
                # tail q-blocks: the exp stream is nearly done, ScalarE has
                # slack -- use it for the PSUM->SBUF copy so the DVE isn't
                # the serializer of the out-proj drain
                nc.scalar.copy(ys[:, sqb - base, :], yps[:, :])
            else:
                nc.vector.tensor_copy(ys[:, sqb - base, :], yps[:, :])
            if sqb == base:
                nc.sync.dma_start(
                    out=yT[et * 128:(et + 1) * 128, base * QB:(base + 2) * QB],
                    in_=ys[:, :, :],
                )
        return thunk

    gctr = [0]
    pace = [1]

    def pop_filler():
        gctr[0] += 1
        pops = 1 if gctr[0] % pace[0] == 0 else 0
        if len(fillers) > 6:
            pops += 1
        if len(fillers) > 10:
            pops += 1
        for _ in range(pops):
            if fillers:
                fillers.popleft()()

    # ---- attention unit for one (head, q-block) ----
    # Each unit's O-matmul flush + softmax normalization is returned as a
    # closure and emitted inside the NEXT unit right after its first exp, so
    # ACT gets the next unit's first exp with no boundary gap.
    def attention(h, qb, leftover):
        pair = h // 2
        ktile, qtile = kT_sb[pair], qT_sb[pair]
        po = (h % 2) * D
        ngr = 2 * (qb + 1)   # groups of 2 j-tiles each
        njt = 4 * (qb + 1)
        oT = oTp.tile([128, QB], F32, tag="oT", name="oT")

        def emit_omms(grp, pT):
            for m in range(2):
                jt = grp * 2 + m
                c0 = 128 * (jt - 4 * qb) if grp >= ngr - 2 else 0
                nc.tensor.matmul(
                    oT[0:D + 1, c0:QB],
                    lhsT=v_sb[jt][:, h, :],
                    rhs=pT[:, m, c0:QB],
                    start=(jt == 0),
                    stop=(jt == njt - 1),
                )

        pend = []
        for grp in range(ngr):
            diag = grp >= ngr - 2
            sT = sTp.tile([128, 2, QB], F32, tag="sT", name="sT")
            for m in range(2):
                jt = grp * 2 + m
                c0 = 128 * (jt - 4 * qb) if diag else 0
                nc.tensor.matmul(
                    sT[:, m, c0:QB],
                    lhsT=ktile[po:po + D, jt * 128:(jt + 1) * 128],
                    rhs=qtile[po:po + D, qb * QB + c0:(qb + 1) * QB],
                    start=True,
                    stop=True,
                )
            pT = pTp.tile([128, 2, QB], BF16, tag="pT", name="pT")
            if diag:
                # one ACT call covering both m; m1's first 128 cols are
                # garbage-exp'd but never read by the O-matmuls
                c0 = 128 * (grp * 2 - 4 * qb)
                nc.scalar.activation(pT[:, :, c0:QB], sT[:, :, c0:QB], AF.Exp)
                # multiplicative causal mask on the diagonal 128-blocks of
                # P = exp(S), applied AFTER exp so the DVE never sits between
                # S-matmul and exp (the O-matmuls trail by 2 groups anyway)
                for m in range(2):
                    jt = grp * 2 + m
                    cm = 128 * (jt - 4 * qb)
                    nc.vector.tensor_mul(
                        pT[:, m, cm:cm + 128], pT[:, m, cm:cm + 128], mask_sb[:, :]
                    )
            else:
                nc.scalar.activation(pT[:, :, :], sT[:, :, :], AF.Exp)
            pend.append((grp, pT))
            if grp == 0 and leftover is not None:
                leftover()
            else:
                pop_filler()
            if len(pend) > 2:
                emit_omms(*pend.pop(0))

        def fin():
            for g0, p0 in pend:
                emit_omms(g0, p0)
            # normalize: att^T = outT[0:D] * (1/Z), Z = outT[D]
            zrow = smallp.tile([1, QB], F32, tag="zrow", name="zrow")
            nc.vector.tensor_copy(zrow[:, :], oT[D:D + 1, :])
            rz = smallp.tile([1, QB], F32, tag="rz", name="rz")
            nc.vector.reciprocal_approx_fast(out=rz[:, :], in_=zrow[:, :])
            zs = smallp.tile([D, QB], F32, tag="zs", name="zs")
            nc.gpsimd.partition_broadcast(zs[:, :], rz[:, :], channels=D)
            nc.vector.tensor_mul(
                attT_sb[pair][po:po + D, qb * QB:(qb + 1) * QB],
                oT[0:D, :],
                zs[:, :],
            )
        return fin

    # pair-0 qb<=1 units run on xT half 0 while preamble B rides as filler
    fin = None
    for qb in (1, 0):
        fin = attention(0, qb, fin)
        fin = attention(1, qb, fin)
    # pair-1 Q/K projection fillers: all of K^T first (every pair-1 S-matmul
    # needs full kT), then Q^T in the pair-1 qb visit order
    for tc4 in range(NQB):
        fillers.extend(mk_qk(3, tc4))
    for tc4 in (3, 2, 1, 0):
        fillers.extend(mk_qk(1, tc4))
    pace[0] = 3
    for qb in (3, 2):
        fin = attention(0, qb, fin)
        fin = attention(1, qb, fin)
    # pair 1 attention carries the out-proj as filler
    pace[0] = 2
    for qb in (3, 2, 1, 0):
        fin = attention(2, qb, fin)
        fin = attention(3, qb, fin)
        for et in range(C // 128):
            fillers.append(mk_outproj(qb, et))
    fin()
    while fillers:
        fillers.popleft()()


def build_nc():
    from contextlib import ExitStack

    nc = bacc.Bacc("TRN2", target_bir_lowering=False)
    xT = nc.dram_tensor("xT", [C, T], BF16, kind="ExternalInput")
    wqkv = nc.dram_tensor("wqkv", [C, 3 * CS], BF16, kind="ExternalInput")
    bqk = nc.dram_tensor("bqk", [128, 4], F32, kind="ExternalInput")
    wp = nc.dram_tensor("wp", [CS, C], BF16, kind="ExternalInput")
    masks = nc.dram_tensor("masks", [128, 128], BF16, kind="ExternalInput")
    yT = nc.dram_tensor("yT", [C, T], BF16, kind="ExternalOutput")
    with tile.TileContext(nc) as tc:
        with nc.allow_low_precision(reason="bf16 matmul inputs; accumulation stays fp32 in PSUM"):
            with ExitStack() as ctx:
                _build_body(nc, tc, ctx, xT, wqkv, bqk, wp, masks, yT)
    nc.compile()
    return nc


def make_masks():
    from ml_dtypes import bfloat16

    r = np.arange(128)[:, None]
    c = np.arange(128)[None, :]
    return np.where(r <= c, 1.0, 0.0).astype(bfloat16)


def make_in_maps(x, W_qkv, b_qkv, W_proj):
    from ml_dtypes import bfloat16

    scale = np.float32(1.0 / np.sqrt(D))
    mask_h = make_masks()
    xT_b = [np.ascontiguousarray(x[b].T).astype(bfloat16) for b in range(B)]
    in_maps = []
    for i in range(NCORES):
        b, g = divmod(i, HPC)
        cs0 = g * CS
        wq = W_qkv[:, cs0:cs0 + CS] * scale
        wk = W_qkv[:, C + cs0:C + cs0 + CS]
        wv = W_qkv[:, 2 * C + cs0:2 * C + cs0 + CS]
        bq = b_qkv[cs0:cs0 + CS] * scale
        bk = b_qkv[C + cs0:C + cs0 + CS]
        in_maps.append({
            "xT": xT_b[b],
            "wqkv": np.concatenate([wq, wk, wv], axis=1).astype(bfloat16),
            "bqk": np.ascontiguousarray(
                np.concatenate([bq, bk]).reshape(4, 128).T
            ).astype(np.float32),
            "wp": np.ascontiguousarray(W_proj[cs0:cs0 + CS, :]).astype(bfloat16),
            "masks": mask_h,
        })
    return in_maps


_NC_CACHE = None


def _get_nc():
    global _NC_CACHE
    if _NC_CACHE is None:
        _NC_CACHE = build_nc()
    return _NC_CACHE


def gather(results, b_qkv, W_proj, b_proj):
    Y = np.zeros((B, T, C), np.float32)
    for i in range(NCORES):
        Y[i // HPC] += results[i]["yT"].astype(np.float32).T
    Y += (b_qkv[2 * C:].astype(np.float32) @ W_proj.astype(np.float32)
          + b_proj.astype(np.float32))[None, None, :]
    return Y


def kernel(x, W_qkv, b_qkv, W_proj, b_proj):
    global LAST_RESULT
    x = np.asarray(x, np.float32)
    W_qkv = np.asarray(W_qkv, np.float32)
    b_qkv = np.asarray(b_qkv, np.float32)
    W_proj = np.asarray(W_proj, np.float32)
    b_proj = np.asarray(b_proj, np.float32)

    nc = _get_nc()
    in_maps = make_in_maps(x, W_qkv, b_qkv, W_proj)
    res = run_bass_kernel_spmd(nc, in_maps, list(range(NCORES)), trace=TRACE)
    LAST_RESULT = res
    if TRACE and res.exec_time_ns is not None:
        print(f"HW exec time: {res.exec_time_ns} ns")
    return gather(res.results, b_qkv, W_proj, b_proj)


# revision 55
# speedup vs baseline: 1.0648x; 1.0393x over previous
"""Causal multi-head attention (B=2, T=2048, C=1024, H=16, d=64) on 8 trn2 cores.

Sharding: core i -> (batch b = i//4, head group g = i%4, 4 heads/core).
Data parallel over B, tensor parallel over heads; the out-proj partial sums
(contraction over this core's 256 channels) are reduced on the host during
the gather step, along with b_proj and the analytically-folded V bias.

Device kernel works entirely in [feature, token] (transposed) layout so no
on-device transposes are needed:
  stage 1: Q^T,K^T = (Wqk)^T x^T   (+bias, 1/sqrt(d) folded into Wq host-side)
           V       = x W_v         (natural layout, used as stage-4 lhsT)
  stage 2: S^T[j,q] = K_h^T.T @ Q_h^T   per head, causal tiles only
  stage 3: P^T = exp(S^T)               (no max subtraction; scores are O(3));
           causal mask applied multiplicatively on the diagonal blocks of P
           AFTER the exp so the DVE mask op is off the S->exp critical path
  stage 4: outT[65,q] = [V_h | 1]^T.T @ P^T  accumulated over j tiles
           (row 64 = softmax denominator Z)
  stage 5: att^T = outT[0:64] * (1/Z) via reciprocal_approx_fast (DVE) +
           partition_broadcast (GPSIMD)
  stage 6: y^T = Wp.T @ att^T  -> DMA out; host sums partials + transposes.

All matmul operands are bfloat16 (full PE rate, fast weight load, half the
SBUF/HBM traffic and PE power of fp32r -> no HAM power throttle); PSUM
accumulation stays fp32.

Schedule: the ScalarE exp stream (~90us) is the long pole of the attention
phase, so the PE work that does NOT feed ACT is woven INTO the attention
phase as filler: preamble computes only V and Q/K for head-pair 0, then
pair-0 attention carries the pair-1 Q/K projection matmuls as fillers and
pair-1 attention carries the out-proj matmuls as fillers.  Input DMA is
split into 512-column chunks so the first projection matmuls start ~3us in.
"""

from collections import deque

import numpy as np

import concourse.bass as bass
import concourse.mybir as mybir
from concourse import bacc
import concourse.tile as tile
from concourse.bass_utils import run_bass_kernel_spmd

B, T, C, H, D = 2, 2048, 1024, 16, 64
NCORES = 8
HPC = 4            # heads per core
CS = HPC * D       # 256 channels per core (per Q/K/V block)
KT = C // 128      # 8 contraction tiles for the projections
NT = T // 128      # 16 token tiles of 128
QB = 512           # query block (psum bank width in fp32)
NQB = T // QB      # 4 query blocks
NEG = -1e9

F32 = mybir.dt.float32
BF16 = mybir.dt.bfloat16

TRACE = False
LAST_RESULT = None


def _build_body(nc, tc, ctx, xT, wqkv, bqk, wp, masks, yT):
    AF = mybir.ActivationFunctionType

    persist = ctx.enter_context(tc.tile_pool(name="persist", bufs=1))

    wqkv_sb = [persist.tile([128, 3 * CS], BF16, tag=f"wqkv{k}", name=f"wqkv{k}") for k in range(KT)]
    bqk_sb = persist.tile([128, 4], F32, tag="bqk", name="bqk_sb")
    wp_sb = [persist.tile([128, C], BF16, tag=f"wp{k}", name=f"wp{k}") for k in range(2)]
    mask_sb = persist.tile([128, 128], BF16, tag="mask", name="mask_sb")
    qT_sb = [persist.tile([128, T], BF16, tag=f"qT{i}", name=f"qT{i}") for i in range(2)]
    kT_sb = [persist.tile([128, T], BF16, tag=f"kT{i}", name=f"kT{i}") for i in range(2)]
    v_sb = [persist.tile([128, HPC, D + 1], BF16, tag=f"v{t}", name=f"v{t}") for t in range(NT)]
    attT_sb = [persist.tile([128, T], BF16, tag=f"attT{i}", name=f"attT{i}") for i in range(2)]
    xT_sb = [persist.tile([128, T], BF16, tag=f"xT{k}", name=f"xT{k}") for k in range(KT)]

    # PSUM: sT 2x2-bank + oT 2x1-bank + fill 2x1-bank = 8 banks
    sTp = ctx.enter_context(tc.tile_pool(name="sT", bufs=2, space="PSUM"))
    oTp = ctx.enter_context(tc.tile_pool(name="outT", bufs=2, space="PSUM"))
    fillp = ctx.enter_context(tc.tile_pool(name="fill", bufs=2, space="PSUM"))
    pTp = ctx.enter_context(tc.tile_pool(name="pT", bufs=12))
    smallp = ctx.enter_context(tc.tile_pool(name="small", bufs=8))
    ysp = ctx.enter_context(tc.tile_pool(name="ystage", bufs=10))

    # ---- DMA in two waves (each dma_start also costs ~650ns of serial
    # SyncE trigger time).  Wave 1 is exactly what the first attention units
    # need: xT columns 0-1023 + the Wq/Wk half of the weights (~4MB instead
    # of 6MB) -- the first exp fires ~8us earlier.  Wave 2 brings Wv (gates
    # the V projection, which the first units' exps cover) and the rest of
    # xT (only needed by qb>=2 / tc4>=2 work that runs much later). ----
    HB = T // 2
    for k in range(KT):
        nc.sync.dma_start(out=xT_sb[k][:, 0:HB], in_=xT[k * 128:(k + 1) * 128, 0:HB])
        nc.sync.dma_start(out=wqkv_sb[k][:, 0:2 * CS], in_=wqkv[k * 128:(k + 1) * 128, 0:2 * CS])
    nc.sync.dma_start(out=bqk_sb[:, :], in_=bqk[:, :])
    nc.sync.dma_start(out=mask_sb[:, :], in_=masks[:, :])
    for k in range(KT):
        nc.sync.dma_start(out=wqkv_sb[k][:, 2 * CS:3 * CS], in_=wqkv[k * 128:(k + 1) * 128, 2 * CS:3 * CS])
    for k in range(KT):
        nc.sync.dma_start(out=xT_sb[k][:, HB:T], in_=xT[k * 128:(k + 1) * 128, HB:T])
    for k in range(2):
        nc.sync.dma_start(out=wp_sb[k][:, :], in_=wp[k * 128:(k + 1) * 128, :])

    ones_f32 = smallp.tile([128, 4], F32, tag="ones_f32", name="ones_f32")
    nc.vector.memset(ones_f32[:, :], 1.0)
    for t in range(NT):
        nc.vector.tensor_copy(v_sb[t][:, :, D], ones_f32[:, :])

    def emit_qk_half(ct, tc4, half, ps):
        # half a k-sweep (4 of 8 contraction tiles) of one Q/K 128x512 block
        for k in range(half * 4, half * 4 + 4):
            nc.tensor.matmul(
                ps[:, :],
                lhsT=wqkv_sb[k][:, ct * 128:(ct + 1) * 128],
                rhs=xT_sb[k][:, tc4 * QB:(tc4 + 1) * QB],
                start=(k == 0),
                stop=(k == KT - 1),
            )
        if half == 1:
            dst = qT_sb[ct] if ct < 2 else kT_sb[ct - 2]
            nc.vector.tensor_scalar_add(
                dst[:, tc4 * QB:(tc4 + 1) * QB], ps[:, :], bqk_sb[:, ct:ct + 1]
            )

    def emit_v(t):
        ps = fillp.tile([128, QB], F32, tag="fps", name="fps")
        for k in range(KT):
            nc.tensor.matmul(
                ps[:, 0:CS],
                lhsT=xT_sb[k][:, t * 128:(t + 1) * 128],
                rhs=wqkv_sb[k][:, 2 * CS:3 * CS],
                start=(k == 0),
                stop=(k == KT - 1),
            )
        nc.vector.tensor_copy(
            v_sb[t][:, :, 0:D],
            ps[:, 0:CS].rearrange("p (h d) -> p h d", h=HPC),
        )

    def mk_qk(ct, tc4):
        def thunk():
            ps = fillp.tile([128, QB], F32, tag="fps", name="fps")
            emit_qk_half(ct, tc4, 0, ps)
            emit_qk_half(ct, tc4, 1, ps)
        return [thunk]

    def mk_v(t):
        def thunk():
            emit_v(t)
        return [thunk]

    # ---- preamble A: just enough for the qb<=1 attention units: Q/K pair 0
    # for tc4 0,1 + V t0-7.  The four Q/K sweeps interleave their k-loops so
    # each arriving (xT[k], wqkv[k]) DMA pair feeds four back-to-back
    # matmuls: the PE never idles waiting for the next k-tile, so the HAM
    # clock-gate warms up during the DMA-paced phase instead of after it.
    pre_sweeps = [(0, 1), (2, 1), (0, 0), (2, 0)]
    pre_ps = [
        fillp.tile([128, QB], F32, tag="fps", name="fps"),
        fillp.tile([128, QB], F32, tag="fps", name="fps"),
        oTp.tile([128, QB], F32, tag="oT", name="oT"),
        oTp.tile([128, QB], F32, tag="oT", name="oT"),
    ]
    for k in range(KT):
        for s, (ct, tc4) in enumerate(pre_sweeps):
            nc.tensor.matmul(
                pre_ps[s][:, :],
                lhsT=wqkv_sb[k][:, ct * 128:(ct + 1) * 128],
                rhs=xT_sb[k][:, tc4 * QB:(tc4 + 1) * QB],
                start=(k == 0),
                stop=(k == KT - 1),
            )
    for s, (ct, tc4) in enumerate(pre_sweeps):
        dst = qT_sb[ct] if ct < 2 else kT_sb[ct - 2]
        nc.vector.tensor_scalar_add(
            dst[:, tc4 * QB:(tc4 + 1) * QB], pre_ps[s][:, :], bqk_sb[:, ct:ct + 1]
        )

    # ---- fillers woven into the attention phase ----
    # preamble B rides as filler during the qb<=1 units: Q/K pair 0 for
    # tc4 3,2 (3 first: the qb=3 units need it next) + V t8-15
    fillers = deque()
    for tc4 in (3, 2):
        for ct in (0, 2):
            fillers.extend(mk_qk(ct, tc4))
    for t in range(8, NT):
        fillers.extend(mk_v(t))

    # out-proj: two q-blocks share one ys staging tile + one output DMA
    # (fewer SyncE triggers, 2KB DMA lines).  qb descends, so slot qb-base+0
    # is written second and fires the DMA.
    ys_hold = {}

    def mk_outproj(sqb, et):
        def thunk():
            yps = fillp.tile([128, QB], F32, tag="fps", name="fps")
            for kc in range(2):
                nc.tensor.matmul(
                    yps[:, :],
                    lhsT=wp_sb[kc][:, et * 128:(et + 1) * 128],
                    rhs=attT_sb[kc][:, sqb * QB:(sqb + 1) * QB],
                    start=(kc == 0),
                    stop=(kc == 1),
                )
            base = 2 if sqb >= 2 else 0
            if sqb - base == 1:
                ys_hold[(et, base)] = ysp.tile([128, 2, QB], BF16, tag="ys", name="ys")
            ys = ys_hold[(et, base)]
            if sqb <= 1 and et % 2 == 0:
                # tail q-blocks: the exp stream is nearly done, ScalarE has
                # slack -- alternate the PSUM->SBUF copies between ScalarE
                # and DVE so neither serializes the out-proj drain
                nc.scalar.copy(ys[:, sqb - base, :], yps[:, :])
            else:
                nc.vector.tensor_copy(ys[:, sqb - base, :], yps[:, :])
            if sqb == base:
                nc.sync.dma_start(
                    out=yT[et * 128:(et + 1) * 128, base * QB:(base + 2) * QB],
                    in_=ys[:, :, :],
                )
        return thunk

    gctr = [0]
    pace = [1]

    def pop_filler():
        if pace[0] == 0:
            return
        gctr[0] += 1
        pops = 1 if gctr[0] % pace[0] == 0 else 0
        if len(fillers) > 6:
            pops += 1
        if len(fillers) > 10:
            pops += 1
        for _ in range(pops):
            if fillers:
                fillers.popleft()()

    # ---- attention unit for one (head, q-block) ----
    # Each unit's O-matmul flush + softmax normalization is returned as a
    # closure and emitted inside the NEXT unit right after its first exp, so
    # ACT gets the next unit's first exp with no boundary gap.
    def attention(h, qb, leftover, defer_all=False):
        pair = h // 2
        ktile, qtile = kT_sb[pair], qT_sb[pair]
        po = (h % 2) * D
        ngr = 2 * (qb + 1)   # groups of 2 j-tiles each
        njt = 4 * (qb + 1)
        oT = oTp.tile([128, QB], F32, tag="oT", name="oT")

        def emit_omms(grp, pT):
            for m in range(2):
                jt = grp * 2 + m
                c0 = 128 * (jt - 4 * qb) if grp >= ngr - 2 else 0
                nc.tensor.matmul(
                    oT[0:D + 1, c0:QB],
                    lhsT=v_sb[jt][:, h, :],
                    rhs=pT[:, m, c0:QB],
                    start=(jt == 0),
                    stop=(jt == njt - 1),
                )

        pend = []
        for grp in range(ngr):
            diag = grp >= ngr - 2
            sT = sTp.tile([128, 2, QB], F32, tag="sT", name="sT")
            for m in range(2):
                jt = grp * 2 + m
                c0 = 128 * (jt - 4 * qb) if diag else 0
                nc.tensor.matmul(
                    sT[:, m, c0:QB],
                    lhsT=ktile[po:po + D, jt * 128:(jt + 1) * 128],
                    rhs=qtile[po:po + D, qb * QB + c0:(qb + 1) * QB],
                    start=True,
                    stop=True,
                )
            pT = pTp.tile([128, 2, QB], BF16, tag="pT", name="pT")
            if diag:
                # one ACT call covering both m; m1's first 128 cols are
                # garbage-exp'd but never read by the O-matmuls
                c0 = 128 * (grp * 2 - 4 * qb)
                nc.scalar.activation(pT[:, :, c0:QB], sT[:, :, c0:QB], AF.Exp)
                # multiplicative causal mask on the diagonal 128-blocks of
                # P = exp(S), applied AFTER exp so the DVE never sits between
                # S-matmul and exp (the O-matmuls trail by 2 groups anyway)
                for m in range(2):
                    jt = grp * 2 + m
                    cm = 128 * (jt - 4 * qb)
                    nc.vector.tensor_mul(
                        pT[:, m, cm:cm + 128], pT[:, m, cm:cm + 128], mask_sb[:, :]
                    )
            else:
                nc.scalar.activation(pT[:, :, :], sT[:, :, :], AF.Exp)
            pend.append((grp, pT))
            if grp == 0 and leftover is not None:
                leftover()
            else:
                pop_filler()
            if not defer_all and len(pend) > 2:
                emit_omms(*pend.pop(0))

        def fin():
            for g0, p0 in pend:
                emit_omms(g0, p0)
            # normalize: att^T = outT[0:D] * (1/Z), Z = outT[D]
            zrow = smallp.tile([1, QB], F32, tag="zrow", name="zrow")
            nc.vector.tensor_copy(zrow[:, :], oT[D:D + 1, :])
            rz = smallp.tile([1, QB], F32, tag="rz", name="rz")
            nc.vector.reciprocal_approx_fast(out=rz[:, :], in_=zrow[:, :])
            zs = smallp.tile([D, QB], F32, tag="zs", name="zs")
            nc.gpsimd.partition_broadcast(zs[:, :], rz[:, :], channels=D)
            nc.vector.tensor_mul(
                attT_sb[pair][po:po + D, qb * QB:(qb + 1) * QB],
                oT[0:D, :],
                zs[:, :],
            )
        return fin

    # The qb=1 units need only wave-1 data, so their S/exp streams start
    # while wave 2 is still in flight; their O-matmuls (which need V) are
    # deferred until after the V t0-7 block, whose PE stall on the Wv DMA is
    # covered by those very exps.
    pace[0] = 0
    fin01 = attention(0, 1, None, defer_all=True)
    fin11 = attention(1, 1, None, defer_all=True)
    for t in range(8):
        emit_v(t)
    fin01()
    pace[0] = 1
    fin = attention(0, 0, fin11)
    fin = attention(1, 0, fin)
    # pair-1 Q/K projection fillers: all of K^T first (every pair-1 S-matmul
    # needs full kT), then Q^T in the pair-1 qb visit order
    for tc4 in range(NQB):
        fillers.extend(mk_qk(3, tc4))
    for tc4 in (3, 2, 1, 0):
        fillers.extend(mk_qk(1, tc4))
    pace[0] = 3
    for qb in (3, 2):
        fin = attention(0, qb, fin)
        fin = attention(1, qb, fin)
    # pair 1 attention carries the out-proj as filler
    pace[0] = 2
    for qb in (3, 2, 1, 0):
        fin = attention(2, qb, fin)
        fin = attention(3, qb, fin)
        for et in range(C // 128):
            fillers.append(mk_outproj(qb, et))
    fin()
    while fillers:
        fillers.popleft()()


def build_nc():
    from contextlib import ExitStack

    nc = bacc.Bacc("TRN2", target_bir_lowering=False)
    xT = nc.dram_tensor("xT", [C, T], BF16, kind="ExternalInput")
    wqkv = nc.dram_tensor("wqkv", [C, 3 * CS], BF16, kind="ExternalInput")
    bqk = nc.dram_tensor("bqk", [128, 4], F32, kind="ExternalInput")
    wp = nc.dram_tensor("wp", [CS, C], BF16, kind="ExternalInput")
    masks = nc.dram_tensor("masks", [128, 128], BF16, kind="ExternalInput")
    yT = nc.dram_tensor("yT", [C, T], BF16, kind="ExternalOutput")
    with tile.TileContext(nc) as tc:
        with nc.allow_low_precision(reason="bf16 matmul inputs; accumulation stays fp32 in PSUM"):
            with ExitStack() as ctx:
                _build_body(nc, tc, ctx, xT, wqkv, bqk, wp, masks, yT)
    nc.compile()
    return nc


def make_masks():
    from ml_dtypes import bfloat16

    r = np.arange(128)[:, None]
    c = np.arange(128)[None, :]
    return np.where(r <= c, 1.0, 0.0).astype(bfloat16)


def make_in_maps(x, W_qkv, b_qkv, W_proj):
    from ml_dtypes import bfloat16

    scale = np.float32(1.0 / np.sqrt(D))
    mask_h = make_masks()
    xT_b = [np.ascontiguousarray(x[b].T).astype(bfloat16) for b in range(B)]
    in_maps = []
    for i in range(NCORES):
        b, g = divmod(i, HPC)
        cs0 = g * CS
        wq = W_qkv[:, cs0:cs0 + CS] * scale
        wk = W_qkv[:, C + cs0:C + cs0 + CS]
        wv = W_qkv[:, 2 * C + cs0:2 * C + cs0 + CS]
        bq = b_qkv[cs0:cs0 + CS] * scale
        bk = b_qkv[C + cs0:C + cs0 + CS]
        in_maps.append({
            "xT": xT_b[b],
            "wqkv": np.concatenate([wq, wk, wv], axis=1).astype(bfloat16),
            "bqk": np.ascontiguousarray(
                np.concatenate([bq, bk]).reshape(4, 128).T
            ).astype(np.float32),
            "wp": np.ascontiguousarray(W_proj[cs0:cs0 + CS, :]).astype(bfloat16),
            "masks": mask_h,
        })
    return in_maps


_NC_CACHE = None


def _get_nc():
    global _NC_CACHE
    if _NC_CACHE is None:
        _NC_CACHE = build_nc()
    return _NC_CACHE


def gather(results, b_qkv, W_proj, b_proj):
    Y = np.zeros((B, T, C), np.float32)
    for i in range(NCORES):
        Y[i // HPC] += results[i]["yT"].astype(np.float32).T
    Y += (b_qkv[2 * C:].astype(np.float32) @ W_proj.astype(np.float32)
          + b_proj.astype(np.float32))[None, None, :]
    return Y


def kernel(x, W_qkv, b_qkv, W_proj, b_proj):
    global LAST_RESULT
    x = np.asarray(x, np.float32)
    W_qkv = np.asarray(W_qkv, np.float32)
    b_qkv = np.asarray(b_qkv, np.float32)
    W_proj = np.asarray(W_proj, np.float32)
    b_proj = np.asarray(b_proj, np.float32)

    nc = _get_nc()
    in_maps = make_in_maps(x, W_qkv, b_qkv, W_proj)
    res = run_bass_kernel_spmd(nc, in_maps, list(range(NCORES)), trace=TRACE)
    LAST_RESULT = res
    if TRACE and res.exec_time_ns is not None:
        print(f"HW exec time: {res.exec_time_ns} ns")
    return gather(res.results, b_qkv, W_proj, b_proj)
